# revision 20
# baseline (speedup 1.0000x reference)
"""Swin-style windowed multi-head attention on 8 Trainium2 NeuronCores.

Problem: nn_Attention_86792699118108
  x [16, 3136, 768], 56x56 spatial, window 14x14 (no padding needed),
  12 heads, head_dim 64. 256 independent windows -> 32 windows per core.

v3 strategy (data-parallel over windows):
  - proj emitted channel-major (out yT [C, tok]): full 128-row outputs,
    7056 PE cycles/window vs 9216; y stored bf16 transposed, host
    un-transposes.
  - v_bias folded into the proj bias on host (softmax rows sum to 1:
    ao_norm = AV(v_hat)/sums + vb exactly).
  - scores for a head PAIR live in one [128,1024] PSUM tile as four
    196-col single-matmul regions at cols {0,256,512,768} ((j,mci) ->
    j*512+mci*256); ONE 4D exp per head pair (6 ACT ops/window).
  - ex/attn/er packed [128, 12, 2, 196]: mci0 rows m=0..127, mci1 rows
    m=128..195 on partitions 0..67; ONE er-multiply per window.
  - AV per head-pair [65, 1024] (j at 512-stride), psa double-buffered;
    sums row evicted by one ACT copy, reciprocal on DVE, gpsimd
    partition-broadcast, per-head normalize muls feed channel-major aoT.
  - evictions balanced: qk->DVE (bias fused via tensor_scalar), v->ACT
    (plain copy), proj->ACT (bias fused via activation Identity).
"""

import numpy as np
import ml_dtypes

WS = 14
NH = 12
HD = 64
C = 768
N = WS * WS  # 196 tokens per window
NCORES = 8

_BF16 = ml_dtypes.bfloat16

_prog_cache = {}


def _rel_index(ws):
    coords = np.stack(np.meshgrid(np.arange(ws), np.arange(ws), indexing="ij"))
    cf = coords.reshape(2, -1)
    rel = (cf[:, :, None] - cf[:, None, :]).transpose(1, 2, 0).astype(np.int64)
    rel[..., 0] += ws - 1
    rel[..., 1] += ws - 1
    rel[..., 0] *= 2 * ws - 1
    return rel.sum(-1)


def _build_program(n_win):
    import concourse.bass as bass
    import concourse.mybir as mybir
    import concourse.tile as tile
    from concourse import bacc
    from contextlib import ExitStack

    assert n_win % 4 == 0
    n_grp = n_win // 4
    n_tok = n_win * N

    BF = mybir.dt.bfloat16
    F32 = mybir.dt.float32
    AF = mybir.ActivationFunctionType

    MC = [(0, 128), (128, 68)]  # key/token chunks within a 196-token window

    nc = bacc.Bacc("TRN2", target_bir_lowering=False, debug=False,
                   num_devices=NCORES)

    x = nc.dram_tensor("x", [n_tok, C], BF, kind="ExternalInput")
    wqkvT = nc.dram_tensor("wqkvT", [C, 3 * C], BF, kind="ExternalInput")
    wpT = nc.dram_tensor("wpT", [C, C], BF, kind="ExternalInput")
    er = nc.dram_tensor("er", [128, NH * 2 * N], BF, kind="ExternalInput")
    qkb = nc.dram_tensor("qkb", [128, 12], F32, kind="ExternalInput")
    pbc = nc.dram_tensor("pbc", [128, 6], F32, kind="ExternalInput")
    y = nc.dram_tensor("y", [C, n_tok], BF, kind="ExternalOutput")

    with ExitStack() as ctx:
        tc = ctx.enter_context(tile.TileContext(nc))
        consts = ctx.enter_context(tc.tile_pool(name="consts", bufs=1))
        grp = ctx.enter_context(tc.tile_pool(name="grp", bufs=2))
        win = ctx.enter_context(tc.tile_pool(name="win", bufs=2))
        # PSUM: 8 banks = pss 1x2 (scores) + psa 2x2 (AV) + psg 2x1 (gemm)
        psg = ctx.enter_context(tc.tile_pool(name="psg", bufs=2, space="PSUM"))
        pss = ctx.enter_context(tc.tile_pool(name="pss", bufs=1, space="PSUM"))
        psa = ctx.enter_context(tc.tile_pool(name="psa", bufs=2, space="PSUM"))

        # ---- constants -------------------------------------------------
        wq = []
        for ic in range(6):
            t = consts.tile([128, 3 * C], BF, tag=f"wq{ic}", name=f"wq{ic}")
            nc.sync.dma_start(out=t, in_=wqkvT[ic * 128:(ic + 1) * 128, :])
            wq.append(t)
        wp = []
        for ic in range(6):
            t = consts.tile([128, C], BF, tag=f"wp{ic}", name=f"wp{ic}")
            nc.sync.dma_start(out=t, in_=wpT[ic * 128:(ic + 1) * 128, :])
            wp.append(t)
        er_t = consts.tile([128, NH * 2 * N], BF, tag="er", name="er_t")
        nc.sync.dma_start(out=er_t, in_=er[:, :])
        qkb_t = consts.tile([128, 12], F32, tag="qkb", name="qkb_t")
        nc.sync.dma_start(out=qkb_t, in_=qkb[:, :])
        pb_t = consts.tile([128, 6], F32, tag="pbc", name="pb_t")
        nc.sync.dma_start(out=pb_t, in_=pbc[:, :])

        def emit_xT(g):
            t0 = g * 4 * N
            xT = []
            for ic in range(6):
                t = grp.tile([128, 4 * N], BF, tag=f"xT{ic}", name=f"xT{ic}")
                nc.sync.dma_start(
                    out=t,
                    in_=x[t0:t0 + 4 * N, ic * 128:(ic + 1) * 128],
                    transpose=True)
                xT.append(t)
            return xT

        def load_thunks(g, xT):
            """qkv emission for group g as thunks (PE filler inside group
            g-1's attention windows)."""
            # 848 = 4*N + 64: zero tail lets the mci1 scores matmul use a
            # 128-col stationary (writes all 128 psum partitions; the
            # garbage rows are zeroed by the er packing and never streamed).
            qk = [grp.tile([128, 848], BF, tag=f"qk{oc}", name=f"qk{oc}")
                  for oc in range(12)]
            v_t = {}
            thunks = []

            def mk_qk(oc):
                def f():
                    ps = [psg.tile([128, 512], F32, tag="gemm", name="psqk")
                          for _ in range(2)]
                    for ic in range(6):
                        for s in range(2):
                            nc.tensor.matmul(
                                ps[s][:, 0:392],
                                wq[ic][:, oc * 128:(oc + 1) * 128],
                                xT[ic][:, s * 392:(s + 1) * 392],
                                start=(ic == 0), stop=(ic == 5))
                    for s in range(2):
                        nc.vector.tensor_scalar_add(
                            qk[oc][:, s * 392:(s + 1) * 392], ps[s][:, 0:392],
                            qkb_t[:, oc:oc + 1])
                    nc.vector.memset(qk[oc][:, 784:848], 0.0)
                return f

            for oc in range(12):
                thunks.append(mk_qk(oc))

            def mk_v(w4, mci):
                mo, msz = MC[mci]
                vt = grp.tile([128, NH * 65], BF,
                              tag=f"v{w4}_{mci}", name=f"v{w4}_{mci}")
                v_t[(w4, mci)] = vt

                def f():
                    vr = vt.rearrange("p (h e) -> p h e", e=65)
                    ps = [psg.tile([128, 512], F32, tag="gemm", name="psv")
                          for _ in range(2)]
                    for ic in range(6):
                        for half in range(2):
                            nc.tensor.matmul(
                                ps[half][:msz, 0:384],
                                xT[ic][:, w4 * N + mo: w4 * N + mo + msz],
                                wq[ic][:, 1536 + half * 384: 1536 + (half + 1) * 384],
                                start=(ic == 0), stop=(ic == 5))
                    # v bias folds into the proj bias host-side; plain copy.
                    for half in range(2):
                        nc.scalar.activation(
                            vr[:msz, half * 6:(half + 1) * 6, 0:64],
                            ps[half][:msz, 0:384]
                            .rearrange("p (h e) -> p h e", e=64),
                            AF.Copy)
                    nc.vector.memset(vr[:msz, :, 64:65], 1.0)
                return f

            for w4 in range(4):
                for mci in range(2):
                    thunks.append(mk_v(w4, mci))
            return qk, v_t, thunks

        def emit_scores(g, w4, qk):
            """scores + exp + er-mul for window w4 -> attn [128, 12, 2, 196]."""
            w0 = w4 * N
            ex = win.tile([128, NH * 2 * N], BF, tag="ex", name="ex")
            exv = ex.rearrange("p (h k n) -> p h k n", k=2, n=N)
            for hg in range(6):
                ps = pss.tile([128, 1024], F32, tag="sc", name="pssc")
                for j in range(2):
                    h = hg * 2 + j
                    ro = (h % 2) * 64
                    for mci, (mo, msz) in enumerate(MC):
                        nc.tensor.matmul(
                            ps[:, j * 512 + mci * 256: j * 512 + mci * 256 + N],
                            qk[6 + h // 2][ro:ro + 64, w0 + mo: w0 + mo + 128],
                            qk[h // 2][ro:ro + 64, w0:w0 + N],
                            start=True, stop=True)
                nc.scalar.activation(
                    exv[:, hg * 2:hg * 2 + 2, :, :],
                    ps.rearrange("p (j k n) -> p j k n", k=2, n=256)
                        [:, :, :, 0:N],
                    AF.Exp)
            attn = win.tile([128, NH * 2 * N], BF, tag="attn", name="attn")
            nc.vector.tensor_mul(attn, ex, er_t)
            return attn.rearrange("p (h k n) -> p h k n", k=2, n=N)

        def emit_av(g, w4, v_t, attn):
            """AV + normalization; returns aoT tiles [128, N] x6."""
            aoT = [win.tile([128, N], BF, tag=f"aoT{i}", name=f"aoT{i}")
                   for i in range(6)]
            for p6 in range(6):
                ps = psa.tile([65, 1024], F32, tag="av", name="psav")
                for j in range(2):
                    h = p6 * 2 + j
                    for mci, (mo, msz) in enumerate(MC):
                        nc.tensor.matmul(
                            ps[:, j * 512:j * 512 + N],
                            v_t[(w4, mci)][:msz, h * 65:(h + 1) * 65],
                            attn[0:msz, h, mci, :],
                            start=(mci == 0), stop=(mci == 1))
                sm = win.tile([1, 2 * N], F32, tag="sums", name="sm")
                nc.scalar.activation(
                    sm.rearrange("p (j n) -> p j n", n=N),
                    ps[64:65].rearrange("p (j n) -> p j n", n=512)[:, :, 0:N],
                    AF.Copy)
                rr = win.tile([1, 2 * N], F32, tag="recr", name="recr")
                nc.vector.reciprocal_approx_fast(rr, sm)
                # only partitions 0..63 are read by the normalize muls
                rrep = win.tile([64, 2 * N], F32, tag="rrep", name="rrep")
                nc.gpsimd.partition_broadcast(rrep, rr, channels=64)
                for j in range(2):
                    nc.vector.tensor_mul(
                        aoT[p6][j * 64:j * 64 + 64, :],
                        ps[0:64, j * 512:j * 512 + N],
                        rrep[0:64, j * N:(j + 1) * N])
            return aoT

        def emit_proj(g, w4, aoT):
            t0 = g * 4 * N + w4 * N
            for oc in range(6):
                ps = psg.tile([128, 512], F32, tag="gemm", name="psp")
                for dc in range(6):
                    nc.tensor.matmul(
                        ps[:, 0:N],
                        wp[dc][:, oc * 128:(oc + 1) * 128],
                        aoT[dc][:, 0:N],
                        start=(dc == 0), stop=(dc == 5))
                yt = win.tile([128, N], BF, tag=f"yt{oc}", name=f"yt{oc}")
                nc.scalar.activation(
                    yt, ps[:, 0:N], AF.Identity, bias=pb_t[:, oc:oc + 1])
                nc.sync.dma_start(
                    out=y[oc * 128:(oc + 1) * 128, t0:t0 + N], in_=yt)

        # ---- software pipeline over groups ------------------------------
        xT0 = emit_xT(0)
        qk_c, vt_c, th0 = load_thunks(0, xT0)
        for th in th0:
            th()
        for g in range(n_grp):
            if g + 1 < n_grp:
                xTn = emit_xT(g + 1)
                qk_n, vt_n, thunks = load_thunks(g + 1, xTn)
            else:
                qk_n, vt_n, thunks = None, None, []
            ti = [0]

            def filler(k):
                for _ in range(k):
                    if ti[0] < len(thunks):
                        thunks[ti[0]]()
                        ti[0] += 1

            for w4 in range(4):
                attn = emit_scores(g, w4, qk_c)
                filler(2)
                aoT = emit_av(g, w4, vt_c, attn)
                filler(2)
                emit_proj(g, w4, aoT)
                filler(1)
            while ti[0] < len(thunks):
                thunks[ti[0]]()
                ti[0] += 1
            qk_c, vt_c = qk_n, vt_n

    nc.compile()
    return nc


def _get_program(n_win):
    if n_win not in _prog_cache:
        _prog_cache[n_win] = _build_program(n_win)
    return _prog_cache[n_win]


def _host_prep(x, qkv_w, q_bias, v_bias, rel_bias_table, proj_w, proj_b, H, W):
    B = x.shape[0]
    nws = H // WS  # windows per side
    xw = (np.asarray(x, np.float32)
          .reshape(B, nws, WS, nws, WS, C)
          .transpose(0, 1, 3, 2, 4, 5)
          .reshape(-1, N, C))  # [Bw, 196, C]

    scale = HD ** -0.5
    wq_s = np.array(qkv_w, np.float32, copy=True)
    wq_s[0:C] *= scale
    wqkvT = np.ascontiguousarray(wq_s.T).astype(_BF16)
    wpT = np.ascontiguousarray(np.asarray(proj_w, np.float32).T).astype(_BF16)

    idx = _rel_index(WS).reshape(-1)
    rpb = np.asarray(rel_bias_table, np.float32)[idx].reshape(N, N, NH)  # [n,m,h]
    erf = np.exp(rpb).transpose(1, 2, 0)  # [m, h, n]
    # pack [128, h, mci, n]: mci0 rows m=0..127; mci1 rows 0..67 = m 128..195
    er_pk = np.zeros((128, NH, 2, N), np.float32)
    er_pk[:, :, 0, :] = erf[0:128]
    er_pk[0:68, :, 1, :] = erf[128:196]
    er = np.ascontiguousarray(er_pk.reshape(128, NH * 2 * N)).astype(_BF16)

    # q/k bias columns (k bias is structurally zero in the reference).
    qkv_b = np.concatenate([
        np.asarray(q_bias, np.float32) * scale,
        np.zeros(C, np.float32)])
    qkb = np.ascontiguousarray(qkv_b.reshape(12, 128).T)

    # v_bias folds into the proj bias exactly (softmax rows sum to 1).
    pb_eff = (np.asarray(proj_b, np.float32)
              + np.asarray(proj_w, np.float32) @ np.asarray(v_bias, np.float32))
    pbc = np.ascontiguousarray(pb_eff.reshape(6, 128).T)

    xbf = np.ascontiguousarray(xw.reshape(-1, C)).astype(_BF16)
    return xbf, wqkvT, wpT, er, qkb, pbc


def _in_maps(prep, ncores):
    xbf, wqkvT, wpT, er, qkb, pbc = prep
    tok_core = xbf.shape[0] // ncores
    return [{
        "x": xbf[c * tok_core:(c + 1) * tok_core],
        "wqkvT": wqkvT, "wpT": wpT, "er": er,
        "qkb": qkb, "pbc": pbc,
    } for c in range(ncores)]


def kernel(x, qkv_w, q_bias, v_bias, rel_bias_table, proj_w, proj_b, H, W,
           _return_results=False):
    from concourse.bass_utils import run_bass_kernel_spmd

    x = np.asarray(x)
    B = x.shape[0]
    H = int(H)
    W = int(W)
    nws = H // WS

    prep = _host_prep(x, qkv_w, q_bias, v_bias, rel_bias_table,
                      proj_w, proj_b, H, W)

    Bw = B * nws * nws
    n_win_core = Bw // NCORES
    nc = _get_program(n_win_core)

    in_maps = _in_maps(prep, NCORES)
    res = run_bass_kernel_spmd(nc, in_maps, list(range(NCORES)))
    # y is [C, tok_core] bf16 per core; un-transpose and window-reverse.
    yw = np.concatenate(
        [np.asarray(res.results[c]["y"], np.float32).T for c in range(NCORES)],
        axis=0)  # [Bw*N, C]
    out = (yw.reshape(B, nws, nws, WS, WS, C)
           .transpose(0, 1, 3, 2, 4, 5)
           .reshape(B, H * W, C).astype(np.float32))
    if _return_results:
        return out, res
    return out
